# revision 21
# baseline (speedup 1.0000x reference)
"""Trainium2 Bass kernel: gamma-scaled negative squared-distance matrix.

Computes out[b,k] = -gamma[k] * (||D[b]||^2 + ||W[k]||^2 - 2*D[b].W[k])
for D [16384,512], W [1000,512], gamma [1000] -> out [16384,1000] fp32.

Strategy
--------
Data-parallel over 8 NeuronCores: D sharded along batch (2048 rows/core),
weights/gamma replicated, no cross-core communication. Per core, 32 psum
groups of [128b x 500k] rotate over 7 banks:

  slots 0-6 (first use of each bank) - baseline aug scheme:
      psum = aug(start=True) + 4x bf16 K=128 matmuls;  DVE tensor_copy out.
      (a virgin PSUM element ignores externally written data on a
      start=False accumulate, so each bank's first group must open with a
      PE start=True write; the aug matmul provides it while also folding
      the -gamma*(dsq+wsq) correction, compensated in split-bf16)
  slots 7-31 - engine-offloaded corrections:
      ScalarE pre-fills the bank with -gamma*wsq (fp32), the 4 matmuls
      accumulate on top with start=False, and the DVE runs one fused
      scalar_tensor_tensor: out = (-gamma_bc * dsq[b]) + psum.

This removes 25 of the 32 aug matmuls from the PE (~8us of its serial
time). dsq/wsq/gamma stay fp32 in the offloaded path, so accuracy is set
by the bf16 cross term (~1e-4).

The device program is raw bacc (hand-written semaphores, no TileContext):
  sync  : input DMAs ordered by first use (aug rows, dt piece 0, wt halves,
          dt piece 1, nc2/gx broadcast tiles, dt rest), then per batch-pair
          1MB output stores with a finely split final pair
  tensor: NWARM warm-up matmuls on scratch bridge the input-DMA window
          (the HAM clock lifts 1.2 -> 2.4 GHz only after a few us of
          sustained matmul activity), then 135 real matmuls
  scalar: psum pre-fill for slots 7+, with a drain before signaling so the
          ACT psum write is fully retired before the PE RMW-accumulates
  vector: per-slot copy/combine psum -> SBUF staging over 3 rotating 1 MB
          buffers
DMA completions are unordered, so every all-of-set dependency uses its own
semaphore (never a prefix-sum wait across independent DMAs).
"""

import os
import sys
import types
from contextlib import ExitStack

sys.path.insert(0, "/opt/trn_rl_repo")

import numpy as np
import ml_dtypes


def _install_ntff_hook():
    """The agent image's ``antenv`` lacks ``axon_hooks``; synthesize it and
    register the ctypes NTFF profile hook so trace=True works (and so a
    BASS_TRACE=1 environment doesn't crash the import in bass_utils)."""
    try:
        import antenv.axon_hooks  # noqa: F401

        return
    except ImportError:
        pass
    try:
        import antenv

        mod = types.ModuleType("antenv.axon_hooks")
        mod._hook = None
        mod.set_axon_ntff_profile_hook = lambda h: setattr(mod, "_hook", h)
        mod.get_axon_ntff_profile_hook = lambda: mod._hook
        sys.modules["antenv.axon_hooks"] = mod
        antenv.axon_hooks = mod
        so = "/opt/axon/libaxon_pjrt.so"
        if os.path.exists(so):
            from trn_agent_boot.trn_boot import _ntff_profile_via_ctypes

            mod._hook = _ntff_profile_via_ctypes(so)
    except Exception:
        pass


_install_ntff_hook()

import concourse.bass as bass  # noqa: E402,F401
from concourse import bacc, mybir  # noqa: E402
from concourse import bass_utils  # noqa: E402

B, F, K = 16384, 512, 1000
NCORES = 8
BS = B // NCORES          # 2048 batch rows per core
P = 128                   # partitions
FC = F // P               # 4 contraction chunks
BT = BS // P              # 16 batch tiles per core
K_TILES = ((0, 500), (500, 500))
NBANK = 7                 # psum banks rotating over groups (+1 for warmup)
NOT = 3                   # output staging buffers
NWARM = 10                # 512-col warm-up matmuls bridging the DMA window

_NC_CACHE = None

# slot order: pair-0 prologue k0-first, then (bi,k0),(bi,k1) per tile
SLOTS = [(0, 0), (1, 0), (0, 1), (1, 1)]
for _bi in range(2, BT):
    SLOTS += [(_bi, 0), (_bi, 1)]


def _build_nc():
    nc = bacc.Bacc("TRN2", target_bir_lowering=False, debug=False)
    bf16 = mybir.dt.bfloat16
    f32 = mybir.dt.float32
    Copy = mybir.ActivationFunctionType.Copy
    Alu = mybir.AluOpType

    dt = nc.dram_tensor("dt", [F, BS], bf16, kind="ExternalInput").ap()
    wt = nc.dram_tensor("wt", [F, K], bf16, kind="ExternalInput").ap()
    # ax = [am | an] aug rows; ngx = [dsq cols | -gamma bcast | -gamma*wsq bcast]
    ax = nc.dram_tensor("ax", [4, BS + K], bf16, kind="ExternalInput").ap()
    ngx = nc.dram_tensor("ngx", [P, BT + 2 * K], f32, kind="ExternalInput").ap()
    o = nc.dram_tensor("o", [BS, K], f32, kind="ExternalOutput").ap()

    dt_v = dt.rearrange("(c p) b -> p c b", p=P)
    wt_v = wt.rearrange("(c p) k -> p c k", p=P)
    o_v = o.rearrange("(t p) k -> p t k", p=P)

    with ExitStack() as ctx:
        dt_sb = ctx.enter_context(nc.sbuf_tensor("dt_sb", [P, FC * BS], bf16)).ap()
        wt_sb = ctx.enter_context(nc.sbuf_tensor("wt_sb", [P, FC * K], bf16)).ap()
        ax_sb = ctx.enter_context(nc.sbuf_tensor("ax_sb", [4, BS + K], bf16)).ap()
        ngx_sb = ctx.enter_context(nc.sbuf_tensor("ngx_sb", [P, BT + 2 * K], f32)).ap()
        warm_in = ctx.enter_context(nc.sbuf_tensor("warm_in", [P, 512], bf16)).ap()
        ots = [
            ctx.enter_context(nc.sbuf_tensor(f"ot{i}", [P, 2 * K], f32)).ap()
            for i in range(NOT)
        ]
        banks = [
            ctx.enter_context(nc.psum_tensor(f"bank{i}", [P, 512], f32)).ap()
            for i in range(NBANK)
        ]
        warm_ps = ctx.enter_context(nc.psum_tensor("warm_ps", [P, 512], f32)).ap()

        s_aux = ctx.enter_context(nc.semaphore("s_aux"))
        s_ngx = ctx.enter_context(nc.semaphore("s_ngx"))
        s_wtc = [ctx.enter_context(nc.semaphore(f"s_wtc{i}")) for i in range(FC)]
        s_q = [ctx.enter_context(nc.semaphore(f"s_q{i}")) for i in range(3)]
        s_ws = ctx.enter_context(nc.semaphore("s_ws"))
        s_pf = ctx.enter_context(nc.semaphore("s_pf"))
        s_mm = ctx.enter_context(nc.semaphore("s_mm"))
        s_cp = ctx.enter_context(nc.semaphore("s_cp"))
        s_ot = [ctx.enter_context(nc.semaphore(f"s_ot{i}")) for i in range(NOT)]

        blk = ctx.enter_context(nc.Block())

        dsq_col = lambda bi: ngx_sb[:, bi : bi + 1]
        ng_bc = ngx_sb[:, BT : BT + K]
        nc2_bc = ngx_sb[:, BT + K :]
        am_sb = ax_sb[:, :BS]
        an_sb = ax_sb[:, BS:]

        @blk.sync
        def _(sync):
            dt3 = dt_sb.rearrange("p (c b) -> p c b", c=FC)
            wt3 = wt_sb.rearrange("p (c k) -> p c k", c=FC)
            # dt pieces sized for packet efficiency: later pieces have longer
            # contiguous dram runs (512B / 1536B / 2048B)
            qsls = [slice(0, 256), slice(256, 1024), slice(1024, BS)]
            sync.dma_start(ax_sb[:], ax[:]).then_inc(s_aux, 16)
            sync.dma_start(wt3[:, 0], wt_v[:, 0]).then_inc(s_wtc[0], 16)
            sync.dma_start(dt3[:, :, qsls[0]], dt_v[:, :, qsls[0]]).then_inc(s_q[0], 16)
            for c in range(1, FC):
                sync.dma_start(wt3[:, c], wt_v[:, c]).then_inc(s_wtc[c], 16)
            sync.dma_start(dt3[:, :, qsls[1]], dt_v[:, :, qsls[1]]).then_inc(s_q[1], 16)
            sync.dma_start(ngx_sb[:], ngx[:]).then_inc(s_ngx, 16)
            sync.dma_start(dt3[:, :, qsls[2]], dt_v[:, :, qsls[2]]).then_inc(s_q[2], 16)
            for pi in range(BT // 2):
                if pi < BT // 2 - 1:
                    sync.wait_ge(s_cp, 4 * (pi + 1))
                    sync.dma_start(
                        o_v[:, 2 * pi : 2 * pi + 2, :], ots[pi % NOT][:]
                    ).then_inc(s_ot[pi % NOT], 16)
                else:
                    # final pair: ever-smaller trailing stores so the last
                    # bytes leave as soon as their combines land
                    sync.wait_ge(s_cp, 4 * pi + 2)
                    sync.dma_start(
                        o_v[:, 2 * pi : 2 * pi + 1, :], ots[pi % NOT][:, :K]
                    ).then_inc(s_ot[pi % NOT], 16)
                    sync.wait_ge(s_cp, 4 * pi + 3)
                    sync.dma_start(
                        o_v[:, 2 * pi + 1 : 2 * pi + 2, :500],
                        ots[pi % NOT][:, K : K + 500],
                    ).then_inc(s_ot[pi % NOT], 16)
                    sync.wait_ge(s_cp, 4 * pi + 4)
                    sync.dma_start(
                        o_v[:, 2 * pi + 1 : 2 * pi + 2, 500:],
                        ots[pi % NOT][:, K + 500 :],
                    ).then_inc(s_ot[pi % NOT], 16)

        @blk.scalar
        def _(scalar):
            # psum pre-fill for slots NBANK.. : bank = -gamma*wsq (fp32)
            scalar.wait_ge(s_ngx, 16)
            for s in range(NBANK, len(SLOTS)):
                bi, ki = SLOTS[s]
                k0, kn = K_TILES[ki]
                scalar.wait_ge(s_cp, s - (NBANK - 1))
                nc.scalar.activation(
                    banks[s % NBANK][:, :kn], nc2_bc[:, k0 : k0 + kn], Copy
                )
                # drain before signaling: the ACT psum write must be fully
                # retired before the PE RMW-accumulates over the bank
                scalar.drain().then_inc(s_pf, 1)

        @blk.tensor
        def _(tensor):
            dt3 = dt_sb.rearrange("p (c b) -> p c b", c=FC)
            wt3 = wt_sb.rearrange("p (c k) -> p c k", c=FC)
            tensor.wait_ge(s_ws, 1)
            for w in range(NWARM):
                nc.tensor.matmul(
                    warm_ps[:],
                    warm_in[:, :P],
                    warm_in[:],
                    start=(w == 0),
                    stop=(w == NWARM - 1),
                )
            tensor.wait_ge(s_q[0], 16)
            tensor.wait_ge(s_aux, 16)

            def emit_aug_group(s, bsl, k0, kn, wt_waits=()):
                bank = banks[s % NBANK]
                nc.tensor.matmul(
                    bank[:, :kn], am_sb[:, bsl], an_sb[:, k0 : k0 + kn],
                    start=True, stop=False,
                )
                for c in range(FC):
                    if c in wt_waits:
                        tensor.wait_ge(s_wtc[c], 16)
                    mmi = nc.tensor.matmul(
                        bank[:, :kn], dt3[:, c, bsl], wt3[:, c, k0 : k0 + kn],
                        start=False, stop=(c == FC - 1),
                    )
                return mmi

            # slots 0-6: aug groups (prologue pair + tile2 + tile3-k0); the
            # first slot waits each wt chunk just before streaming it
            bsl_of = lambda bi: slice(bi * P, (bi + 1) * P)
            emit_aug_group(0, bsl_of(0), *K_TILES[0], wt_waits=range(FC)).then_inc(
                s_mm, 1
            )
            emit_aug_group(1, bsl_of(1), *K_TILES[0]).then_inc(s_mm, 1)
            emit_aug_group(2, bsl_of(0), *K_TILES[1]).then_inc(s_mm, 1)
            emit_aug_group(3, bsl_of(1), *K_TILES[1]).then_inc(s_mm, 1)
            tensor.wait_ge(s_q[1], 16)
            emit_aug_group(4, bsl_of(2), *K_TILES[0]).then_inc(s_mm, 1)
            emit_aug_group(5, bsl_of(2), *K_TILES[1]).then_inc(s_mm, 1)
            emit_aug_group(6, bsl_of(3), *K_TILES[0]).then_inc(s_mm, 1)
            # slot 7 on: pre-filled banks, pure accumulate groups
            s = NBANK
            while s < len(SLOTS):
                bi, ki = SLOTS[s]
                if ki == 0 and bi == 8:
                    tensor.wait_ge(s_q[2], 16)
                if ki == 0 and s + 1 < len(SLOTS) and SLOTS[s + 1] == (bi, 1):
                    # interleave the tile's two k-half groups
                    tensor.wait_ge(s_pf, s + 2 - NBANK)
                    b0, b1 = banks[s % NBANK], banks[(s + 1) % NBANK]
                    for c in range(FC):
                        for ki2, (k0, kn) in enumerate(K_TILES):
                            mmi = nc.tensor.matmul(
                                (b0 if ki2 == 0 else b1)[:, :kn],
                                dt3[:, c, bsl_of(bi)],
                                wt3[:, c, k0 : k0 + kn],
                                start=False,
                                stop=(c == FC - 1),
                            )
                            if c == FC - 1:
                                mmi.then_inc(s_mm, 1)
                    s += 2
                else:
                    # lone group (slot 7 = tile3-k1)
                    k0, kn = K_TILES[ki]
                    tensor.wait_ge(s_pf, s + 1 - NBANK)
                    bank = banks[s % NBANK]
                    for c in range(FC):
                        mmi = nc.tensor.matmul(
                            bank[:, :kn], dt3[:, c, bsl_of(bi)],
                            wt3[:, c, k0 : k0 + kn],
                            start=False, stop=(c == FC - 1),
                        )
                    mmi.then_inc(s_mm, 1)
                    s += 1

        @blk.vector
        def _(vector):
            nc.vector.memset(warm_in[:], 0.0).then_inc(s_ws, 1)
            for s, (bi, ki) in enumerate(SLOTS):
                k0, kn = K_TILES[ki]
                pi, sub = bi // 2, bi % 2
                ot = ots[pi % NOT]
                if s == NBANK:
                    vector.wait_ge(s_ngx, 16)
                vector.wait_ge(s_mm, s + 1)
                if pi >= NOT and ki == 0 and sub == 0:
                    # staging buffer reuse: wait store of pair pi-NOT
                    vector.wait_ge(s_ot[pi % NOT], 16 * (pi // NOT))
                dst = ot[:, sub * K + k0 : sub * K + k0 + kn]
                if s < NBANK:
                    nc.vector.tensor_copy(dst, banks[s % NBANK][:, :kn]).then_inc(
                        s_cp, 1
                    )
                else:
                    nc.vector.scalar_tensor_tensor(
                        dst,
                        ng_bc[:, k0 : k0 + kn],
                        dsq_col(bi),
                        banks[s % NBANK][:, :kn],
                        Alu.mult,
                        Alu.add,
                    ).then_inc(s_cp, 1)

    nc.compile()
    return nc


def _get_nc():
    global _NC_CACHE
    if _NC_CACHE is None:
        _NC_CACHE = _build_nc()
    return _NC_CACHE


def _prep_in_maps(D, weight, gamma):
    D = np.asarray(D, dtype=np.float32)
    weight = np.asarray(weight, dtype=np.float32)
    gamma = np.asarray(gamma, dtype=np.float32)

    bf16 = ml_dtypes.bfloat16
    DT = np.ascontiguousarray(D.T).astype(bf16)                  # [F, B]
    WT2 = np.ascontiguousarray((2.0 * gamma[:, None] * weight).T).astype(bf16)
    d_sq = np.square(D, dtype=np.float64).sum(axis=1).astype(np.float32)
    w_sq = np.square(weight, dtype=np.float64).sum(axis=1)

    # Compensated bf16 augmentation rows for the first-use (slot 0-6) groups:
    #   psum aug = -gamma*(w_sq + d_sq) via [1,1,r_hi,r_lo]^T.[c_hi,c_lo,-gb,-gb]
    gb = gamma.astype(bf16).astype(np.float32)
    c = (-gamma.astype(np.float64) * (w_sq + 512.0)).astype(np.float32)
    c_hi = c.astype(bf16).astype(np.float32)
    c_lo = c - c_hi
    r = d_sq - 512.0
    r_hi = r.astype(bf16).astype(np.float32)
    r_lo = r - r_hi
    AM = np.stack(
        [np.ones(B, np.float32), np.ones(B, np.float32), r_hi, r_lo]
    ).astype(bf16)
    AN = np.stack([c_hi, c_lo, -gb, -gb]).astype(bf16)

    nc2_row = (-gamma.astype(np.float64) * w_sq).astype(np.float32)  # [K]

    in_maps = []
    for ci in range(NCORES):
        sl = slice(ci * BS, (ci + 1) * BS)
        dsq_cols = d_sq[sl].reshape(BT, P).T                         # [P, BT]
        NGX = np.concatenate(
            [
                dsq_cols,
                np.broadcast_to(-gamma, (P, K)),
                np.broadcast_to(nc2_row, (P, K)),
            ],
            axis=1,
        ).astype(np.float32)
        AX = np.concatenate([AM[:, sl], AN], axis=1)
        in_maps.append(
            {
                "dt": np.ascontiguousarray(DT[:, sl]),
                "wt": WT2,
                "ax": np.ascontiguousarray(AX),
                "ngx": np.ascontiguousarray(NGX),
            }
        )
    return in_maps


def kernel_with_results(D, weight, gamma, trace=False):
    """Run on 8 cores; returns (full_output, BassKernelResults)."""
    nc = _get_nc()
    in_maps = _prep_in_maps(D, weight, gamma)
    res = bass_utils.run_bass_kernel_spmd(
        nc, in_maps, core_ids=list(range(NCORES)), trace=trace
    )
    out = np.concatenate([r["o"] for r in res.results], axis=0)
    return out, res


def kernel(D, weight, gamma):
    out, _ = kernel_with_results(D, weight, gamma)
    return out
